# revision 18
# baseline (speedup 1.0000x reference)
"""Bilinear resampling (tf-resampler semantics) on 8 TRN2 NeuronCores.

out[b,y,x] = bilinear_sample(imgs[b], y + dvfs[b,y,x,1], x + dvfs[b,y,x,0])
with zero-padding for out-of-bounds corners.

Strategy: pure data-parallel over batch (4 images per core).  On-chip the
per-pixel 2D gather is computed as a dense separable "hat" select-sum:

    out[y,x] = sum_r hat(dy-r) * sum_c hat(dx-c) * I[y+r, x+c]

where hat(t) = max(0, 1-|t|).  hat(dy-r)*hat(dx-c) is exactly the bilinear
corner weight for corner (y+r, x+c) and is nonzero only for the 4 corners
of each pixel, so summing over a tap set that covers all occurring
(floor(dy), floor(dx)) pairs is exact.  The tap set is computed on the host
from the actual displacement field (cheap histogram) PER 128-ROW TILE SLOT
(union across the 8 cores so one SPMD program serves all), so device work
scales with the true local support of the data.

v2 layout changes vs the first working version:
- fp16 end-to-end: the host pads images to fp16 and de-interleaves dvfs
  into fp16 dx/dy planes; the kernel computes taps in fp16 (2x DVE mode)
  and writes fp16 output (host upcasts).  This kills the on-chip
  fp32->bf16 conversion pass entirely, halves all DMA traffic, and fp16's
  11-bit mantissa is far more accurate than bf16 for this data range.
- The output accumulator is fp16 (not fp32), so the vertical blend ops
  also run in the DVE 2x perf mode.
- Per-tile-slot adaptive tap sets and row windows.
- Images are zero-padded on the host; each 128-row tile loads all n_r
  row-shifted copies of the image window with ONE overlapping 3D-AP DMA
  (engine APs require partition base in {0,32,64,96}, so row shifts cannot
  be partition-offset views; DMA has no such restriction).
- Hat coefficient planes are built on the Scalar (ACT) engine; tap
  multiply/adds run on the Vector (DVE) engine.
"""

import sys

sys.path.insert(0, "/opt/trn_rl_repo")

import dataclasses
from contextlib import ExitStack

import numpy as np

import concourse.bass as bass
import concourse.mybir as mybir
from concourse import tile
from concourse.bass_utils import run_bass_kernel_spmd

F32 = mybir.dt.float32
F16 = mybir.dt.float16
AF = mybir.ActivationFunctionType

N_CORES = 8


def _tap_sets(dvx, dvy):
    """Exact tap support from the data: {r: (c0, c1)} for taps (r, c) such
    that some pixel has floor(dy) in {r-1, r} and floor(dx) in {c-1, c}."""
    fx = np.floor(dvx).astype(np.int64).ravel()
    fy = np.floor(dvy).astype(np.int64).ravel()
    lo = int(min(fx.min(), fy.min()))
    hi = int(max(fx.max(), fy.max()))
    n = hi - lo + 1
    joint = np.bincount((fy - lo) * n + (fx - lo), minlength=n * n) > 0
    joint = joint.reshape(n, n)
    # tap (r, c) needed iff joint[r - dr, c - dc] for (dr, dc) in {0,1}^2
    need = np.zeros((n + 1, n + 1), dtype=bool)
    for dr in (0, 1):
        for dc in (0, 1):
            need[dr : dr + n, dc : dc + n] |= joint
    taps = {}
    for ri in range(n + 1):
        cs = np.nonzero(need[ri])[0]
        if len(cs):
            taps[ri + lo] = (int(cs.min()) + lo, int(cs.max()) + lo)
    return taps


def _split_multi_waits(nc):
    """This stack's walrus accepts at most one sync-wait per instruction;
    Tile emits several.  Hoist all-but-one wait onto preceding NoOps on the
    same engine queue (sequential execution makes that equivalent)."""
    for fn in nc.m.functions:
        for blk in fn.blocks:
            new_insts = []
            for ins in blk.instructions:
                si = ins.sync_info
                if si is not None and si.on_wait and len(si.on_wait) > 1:
                    waits = list(si.on_wait)
                    for w in waits[:-1]:
                        new_insts.append(
                            mybir.InstNoOp(
                                name=nc.get_next_instruction_name(),
                                engine=ins.engine,
                                bass_nofuse=True,
                                sync_info=mybir.SyncInfo(
                                    on_wait=[w], on_update=[]
                                ),
                            )
                        )
                    si.on_wait = [waits[-1]]
                new_insts.append(ins)
            blk.instructions = new_insts


def _plan(slot_taps, H, W):
    """Derive global padding and per-slot layouts from the per-slot tap sets.

    Returns (pads, slots) where slots[slot_key] holds the per-slot tap plan.
    """
    rmin_g = min(min(t.keys()) for t in slot_taps.values())
    rmax_g = max(max(t.keys()) for t in slot_taps.values())
    cmin_g = min(c0 for t in slot_taps.values() for c0, _ in t.values())
    cmax_g = max(c1 for t in slot_taps.values() for _, c1 in t.values())
    pad_t, pad_b = max(0, -rmin_g), max(0, rmax_g)
    pad_l, pad_r = max(0, -cmin_g), max(0, cmax_g)
    Wp = W + pad_l + pad_r
    if Wp % 2 == 1:
        pad_r += 1
        Wp += 1

    slots = {}
    for key, taps in slot_taps.items():
        rs = sorted(taps.keys())
        rmin, rmax = rs[0], rs[-1]
        n_r = rmax - rmin + 1
        c_range = {}
        for r in rs:
            c0, c1 = taps[r]
            # keep every tap view 4-byte aligned so the DVE 2x perf mode
            # engages: (pad_l + c0) must be even
            if (pad_l + c0) % 2 != 0:
                c0 -= 1
            c_range[r] = (c0, c1)
        cs_lo = min(c0 for c0, _ in c_range.values())
        cs_hi = max(c1 for _, c1 in c_range.values())
        cs_union = list(range(cs_lo, cs_hi + 1))
        slots[key] = dict(
            rs=rs,
            rmin=rmin,
            n_r=n_r,
            c_range=c_range,
            cs_union=cs_union,
            c_idx={c: i for i, c in enumerate(cs_union)},
        )
    return (pad_t, pad_b, pad_l, pad_r, Wp), slots


def _build(slot_taps, n_imgs, H, W, repeat=1, walrus=True, ablate=()):
    """ablate: subset of {'hats','taps','ibdma'} replacing that component
    with cheap stand-ins (wrong numerics) — used for critical-path probes."""
    pads_all, slots = _plan(slot_taps, H, W)
    pad_t, pad_b, pad_l, pad_r, Wp = pads_all
    Hp = H + pad_t + pad_b

    nr_max = max(s["n_r"] for s in slots.values())
    kb_max = max(len(s["cs_union"]) for s in slots.values())
    kt_max = max(
        max(c1 - c0 + 1 for c0, c1 in s["c_range"].values()) for s in slots.values()
    )

    nc = bass.Bass()
    imgs = nc.dram_tensor("imgs", [n_imgs, Hp, Wp], F16, kind="ExternalInput")
    dvx = nc.dram_tensor("dvx", [n_imgs, H, W], F16, kind="ExternalInput")
    dvy = nc.dram_tensor("dvy", [n_imgs, H, W], F16, kind="ExternalInput")
    out = nc.dram_tensor("out", [n_imgs, H, W], F16, kind="ExternalOutput")

    # activation() biases must come from the const-AP registry
    all_rc = set()
    for s in slots.values():
        all_rc |= set(s["rs"]) | set(s["cs_union"])
    consts = sorted({-float(v) for v in all_rc} - {0.0, 1.0})
    for v in consts:
        t = nc.alloc_sbuf_tensor(f"const-f32-{v}", [128, 1], F32)
        nc.gpsimd.memset(t.ap(), v)
        nc.const_aps.aps[(F32, v)] = t.ap()
    hat_static = None
    if "hats" in ablate:
        hs = nc.alloc_sbuf_tensor(
            "hat-static", [128, max(kb_max, nr_max) * W], F16
        )
        nc.gpsimd.memset(hs.ap(), 0.5)
        hat_static = hs.ap()
    nc.all_engine_barrier()

    with ExitStack() as ctx:
        tc = ctx.enter_context(tile.TileContext(nc))
        img_pool = ctx.enter_context(tc.tile_pool(name="img", bufs=2))
        dx_pool = ctx.enter_context(tc.tile_pool(name="dx", bufs=2))
        dy_pool = ctx.enter_context(tc.tile_pool(name="dy", bufs=2))
        bp_pool = ctx.enter_context(tc.tile_pool(name="bp", bufs=2))
        av_pool = ctx.enter_context(tc.tile_pool(name="av", bufs=1))
        t_pool = ctx.enter_context(tc.tile_pool(name="t", bufs=1))
        tf_pool = ctx.enter_context(tc.tile_pool(name="tf", bufs=1))

        def emit_body():
            for b, t0 in [
                (bb, tt) for bb in range(n_imgs) for tt in range(0, H, 128)
            ]:
                emit_slot(b, t0)

        def emit_slot(b, t0):
            s = slots[(b, t0)]
            rs, rmin, n_r = s["rs"], s["rmin"], s["n_r"]
            c_range, cs_union, c_idx = s["c_range"], s["cs_union"], s["c_idx"]
            kb = len(cs_union)

            # all n_r row-shifted image windows in one overlapping DMA:
            # IB[p, j*Wp + u] = imgs_padded[b, t0 + rmin + pad_t + j + p, u]
            IB = img_pool.tile([128, nr_max * Wp], F16, tag="IB")
            n_r_dma = 1 if "ibdma" in ablate else n_r
            src = dataclasses.replace(
                imgs[b],
                ap=[[Wp, 128], [Wp, n_r_dma], [1, Wp]],
                offset=b * Hp * Wp + (t0 + rmin + pad_t) * Wp,
            )
            dst = IB[:, 0 : n_r_dma * Wp].rearrange("p (j w) -> p j w", j=n_r_dma)
            nc.sync.dma_start(out=dst, in_=src)

            DX = dx_pool.tile([128, W], F16, tag="DX")
            nc.sync.dma_start(out=DX[:, :], in_=dvx[b, t0 : t0 + 128, :])
            DY = dy_pool.tile([128, W], F16, tag="DY")
            nc.sync.dma_start(out=DY[:, :], in_=dvy[b, t0 : t0 + 128, :])

            # horizontal hat planes: B_c = relu(1 - |dx - c|); the |.| step
            # is per-c (bias differs), the affine+relu is ONE batched
            # in-place ACT op over the whole stack
            if "hats" in ablate:
                BP = hat_static
            else:
                BP = bp_pool.tile([128, kb_max * W], F16, tag="BP")
                for c in cs_union:
                    i = c_idx[c]
                    nc.scalar.activation(
                        BP[:, i * W : (i + 1) * W],
                        DX[:, :],
                        AF.Abs,
                        bias=-float(c),
                        scale=1.0,
                    )
                nc.scalar.activation(
                    BP[:, 0 : kb * W],
                    BP[:, 0 : kb * W],
                    AF.Relu,
                    bias=1.0,
                    scale=-1.0,
                )

            if "taps" in ablate:
                TF = tf_pool.tile([128, nr_max * W], F16, tag="TF")
                nc.vector.tensor_mul(TF[:, 0:W], DX[:, :], DY[:, :])
                nc.sync.dma_start(out=out[b, t0 : t0 + 128, :], in_=TF[:, 0:W])
                return

            # vertical hat stack: A_r = relu(1 - |dy - r|), same batching
            if "hats" in ablate:
                AVS = hat_static
            else:
                AVS = av_pool.tile([128, nr_max * W], F16, tag="AVS")
                for r in rs:
                    j = r - rmin
                    nc.scalar.activation(
                        AVS[:, j * W : (j + 1) * W],
                        DY[:, :],
                        AF.Abs,
                        bias=-float(r),
                        scale=1.0,
                    )
                nc.scalar.activation(
                    AVS[:, 0 : n_r * W],
                    AVS[:, 0 : n_r * W],
                    AF.Relu,
                    bias=1.0,
                    scale=-1.0,
                )

            # group consecutive rows with identical c-ranges: one product
            # instruction + one shared fold tree per group (fewer, larger
            # DVE instructions; same element count)
            TF = tf_pool.tile([128, nr_max * W], F16, tag="TF")
            gk_cap = max(kt_max, 16)
            groups = []
            for r in rs:
                c0, c1 = c_range[r]
                k = c1 - c0 + 1
                if (
                    groups
                    and groups[-1][1] == (c0, c1)
                    and groups[-1][0][-1] == r - 1
                    and (len(groups[-1][0]) + 1) * k <= gk_cap
                ):
                    groups[-1][0].append(r)
                else:
                    groups.append(([r], (c0, c1)))

            for g_rs, (c0, c1) in groups:
                g = len(g_rs)
                j0 = 0 if "ibdma" in ablate else g_rs[0] - rmin
                jf0 = g_rs[0] - rmin
                k = c1 - c0 + 1
                row_step = 0 if "ibdma" in ablate else Wp
                T = t_pool.tile([128, gk_cap * W], F16, tag="T")
                iv = dataclasses.replace(
                    IB[:, :],
                    ap=[[nr_max * Wp, 128], [row_step, g], [1, k], [1, W]],
                    offset=j0 * Wp + pad_l + c0,
                )
                bp = dataclasses.replace(
                    BP[:, :],
                    ap=[[kb_max * W, 128], [0, g], [W, k], [1, W]],
                    offset=c_idx[c0] * W,
                )
                tf_slots = TF[:, jf0 * W : (jf0 + g) * W].rearrange(
                    "p (g w) -> p g w", g=g
                )
                if k == 1:
                    nc.vector.tensor_tensor(
                        tf_slots,
                        dataclasses.replace(
                            iv, ap=[[nr_max * Wp, 128], [row_step, g], [1, W]]
                        ),
                        dataclasses.replace(
                            bp, ap=[[kb_max * W, 128], [0, g], [1, W]]
                        ),
                        mybir.AluOpType.mult,
                    )
                    continue
                tv = T[:, 0 : g * k * W].rearrange(
                    "p (g k w) -> p g k w", g=g, k=k
                )
                nc.vector.tensor_tensor(tv, iv, bp, mybir.AluOpType.mult)
                # shared fold tree along k (row stride in T stays k*W)
                kk = k

                def tview(koff, kn, kg=g, kw=k):
                    if kn == 1:
                        return dataclasses.replace(
                            T[:, :],
                            ap=[[gk_cap * W, 128], [kw * W, kg], [1, W]],
                            offset=koff * W,
                        )
                    return dataclasses.replace(
                        T[:, :],
                        ap=[[gk_cap * W, 128], [kw * W, kg], [W, kn], [1, W]],
                        offset=koff * W,
                    )

                while kk > 2:
                    if kk % 2 == 1:
                        nc.vector.tensor_add(
                            tview(0, 1), tview(0, 1), tview(kk - 1, 1)
                        )
                        kk -= 1
                    else:
                        h = kk // 2
                        nc.vector.tensor_add(tview(0, h), tview(0, h), tview(h, h))
                        kk = h
                nc.vector.tensor_add(tf_slots, tview(0, 1), tview(1, 1))

            # vertical blend, batched: TF *= AVS (one op), then fold rows
            m = n_r
            nc.vector.tensor_tensor(
                TF[:, 0 : m * W],
                TF[:, 0 : m * W],
                AVS[:, 0 : m * W],
                mybir.AluOpType.mult,
            )
            while m > 1:
                if m % 2 == 1:
                    nc.vector.tensor_add(
                        TF[:, 0:W], TF[:, 0:W], TF[:, (m - 1) * W : m * W]
                    )
                    m -= 1
                else:
                    h = m // 2
                    nc.vector.tensor_add(
                        TF[:, 0 : h * W],
                        TF[:, 0 : h * W],
                        TF[:, h * W : m * W],
                    )
                    m = h

            nc.sync.dma_start(out=out[b, t0 : t0 + 128, :], in_=TF[:, 0:W])

        for _ in range(repeat):
            emit_body()

    if walrus:
        _split_multi_waits(nc)
    return nc, (pad_t, pad_b, pad_l, pad_r, Wp)


def _make_runner(nc):
    """Mirror of bass2jax.run_bass_via_pjrt's multi-core path, but returning
    a reusable jitted callable so the NEFF can be re-executed for timing."""
    import jax
    from jax.experimental.shard_map import shard_map
    from jax.sharding import Mesh, PartitionSpec

    from concourse import bass2jax, mybir as mb

    bass2jax.install_neuronx_cc_hook()
    partition_name = nc.partition_id_tensor.name if nc.partition_id_tensor else None
    in_names, out_names, out_avals, zero_outs = [], [], [], []
    for alloc in nc.m.functions[0].allocations:
        if not isinstance(alloc, mb.MemoryLocationSet):
            continue
        name = alloc.memorylocations[0].name
        if alloc.kind == "ExternalInput":
            if name != partition_name:
                in_names.append(name)
        elif alloc.kind == "ExternalOutput":
            out_names.append(name)
            shape = tuple(alloc.tensor_shape)
            dtype = mb.dt.np(alloc.dtype)
            out_avals.append(jax.core.ShapedArray(shape, dtype))
            zero_outs.append(np.zeros(shape, dtype))
    n_params = len(in_names)
    n_outs = len(out_avals)
    all_in_names = list(in_names) + list(out_names)
    if partition_name is not None:
        all_in_names.append(partition_name)

    def _body(*args):
        operands = list(args)
        if partition_name is not None:
            operands.append(bass2jax.partition_id_tensor())
        outs = bass2jax._bass_exec_p.bind(
            *operands,
            out_avals=tuple(out_avals),
            in_names=tuple(all_in_names),
            out_names=tuple(out_names),
            lowering_input_output_aliases=(),
            sim_require_finite=True,
            sim_require_nnan=True,
            nc=nc,
        )
        return tuple(outs)

    devices = jax.devices()[:N_CORES]
    mesh = Mesh(np.asarray(devices), ("core",))
    in_specs = (PartitionSpec("core"),) * (n_params + n_outs)
    out_specs = (PartitionSpec("core"),) * n_outs
    # no donation: the kernel writes every output element, so the "zero"
    # output buffers can be staged on device once and reused across calls
    sharded = jax.jit(
        shard_map(
            _body, mesh=mesh, in_specs=in_specs, out_specs=out_specs, check_rep=False
        ),
        keep_unused=True,
    )

    from jax.sharding import NamedSharding

    shd = NamedSharding(mesh, PartitionSpec("core"))

    def run(in_maps, materialize=True, _staged={}):
        key = id(in_maps)
        if key not in _staged:
            per_core = [[np.asarray(m[name]) for name in in_names] for m in in_maps]
            concat_in = [
                np.concatenate([per_core[c][i] for c in range(N_CORES)], axis=0)
                for i in range(n_params)
            ]
            concat_zeros = [
                np.zeros((N_CORES * z.shape[0], *z.shape[1:]), z.dtype)
                for z in zero_outs
            ]
            _staged.clear()
            _staged[key] = [
                jax.device_put(a, shd) for a in concat_in + concat_zeros
            ]
            jax.block_until_ready(_staged[key])
        args = _staged[key]
        out_arrs = sharded(*args)
        jax.block_until_ready(out_arrs)
        if not materialize:
            return None
        return [
            {
                name: np.asarray(out_arrs[i]).reshape(N_CORES, *out_avals[i].shape)[c]
                for i, name in enumerate(out_names)
            }
            for c in range(N_CORES)
        ]

    return run


def _null_runner():
    """Tiny copy kernel used to measure fixed per-call dispatch overhead."""
    nc = bass.Bass()
    x = nc.dram_tensor("x", [128, 128], F32, kind="ExternalInput")
    y = nc.dram_tensor("y", [128, 128], F32, kind="ExternalOutput")
    from contextlib import ExitStack

    with ExitStack() as ctx:
        tc = ctx.enter_context(tile.TileContext(nc))
        pool = ctx.enter_context(tc.tile_pool(name="p", bufs=1))
        t = pool.tile([128, 128], F32)
        nc.sync.dma_start(out=t[:, :], in_=x[:, :])
        nc.sync.dma_start(out=y[:, :], in_=t[:, :])
    _split_multi_waits(nc)
    runner = _make_runner(nc)
    in_maps = [{"x": np.zeros((128, 128), np.float32)} for _ in range(N_CORES)]
    return runner, in_maps


def _prepare(imgs, dvfs, repeat=1):
    imgs = np.asarray(imgs)
    dvfs = np.asarray(dvfs, dtype=np.float32)
    B, H, W = imgs.shape[0], imgs.shape[1], imgs.shape[2]
    n_per = B // N_CORES

    dvx = np.ascontiguousarray(dvfs[..., 0])
    dvy = np.ascontiguousarray(dvfs[..., 1])

    # per-tile-slot tap sets, unioned across the 8 cores (SPMD: one program)
    slot_taps = {}
    for bl in range(n_per):
        for t0 in range(0, H, 128):
            idx = [c * n_per + bl for c in range(N_CORES)]
            slot_taps[(bl, t0)] = _tap_sets(
                dvx[idx, t0 : t0 + 128], dvy[idx, t0 : t0 + 128]
            )

    nc, pads = _build(slot_taps, n_per, H, W, repeat=repeat)
    pad_t, pad_b, pad_l, pad_r, Wp = pads
    imgs_p = np.zeros((B, H + pad_t + pad_b, Wp), np.float16)
    imgs_p[:, pad_t : pad_t + H, pad_l : pad_l + W] = imgs.reshape(B, H, W)
    dvx16 = dvx.astype(np.float16)
    dvy16 = dvy.astype(np.float16)
    in_maps = [
        {
            "imgs": imgs_p[i * n_per : (i + 1) * n_per],
            "dvx": dvx16[i * n_per : (i + 1) * n_per],
            "dvy": dvy16[i * n_per : (i + 1) * n_per],
        }
        for i in range(N_CORES)
    ]
    return nc, in_maps, (B, H, W)


def _run(imgs, dvfs):
    nc, in_maps, (B, H, W) = _prepare(imgs, dvfs)
    runner = _make_runner(nc)
    results = runner(in_maps)
    outs = [np.asarray(m["out"]) for m in results]
    full = np.concatenate(outs, axis=0).reshape(B, H, W, 1).astype(np.float32)
    return full, runner, in_maps


def kernel(**inputs):
    full, _, _ = _run(inputs["imgs"], inputs["dvfs"])
    return full
